# revision 11
# baseline (speedup 1.0000x reference)
"""Multi-head attention w/ KV cache, tensor-parallel over 8 TRN2 NeuronCores.

Sharding: heads are split 2-per-core (W_Q/W_K/W_V column shards, KV cache head
shards).  Each core computes Q/K/V projections for its 2 heads, full attention
over the 4096-key axis (2048 cache + 2048 new), then two AllToAlls convert the
head-sharded attention output O^T into a token-sharded full-depth O^T so every
core computes W_O for its own 512-token shard — no all-reduce and no
core-dependent control flow (pure SPMD).

Layout notes (everything transposed so the PE contracts over partitions):
  x^T        [1024, 4096]  d_model on partitions, tokens b-major on free axis
  Q^T/K^T    [128, 4096]   2 heads stacked on partitions (64 rows each)
  scores^T   [keys, q]     K^T stationary, Q^T moving, 2 heads row-packed
  P = exp(s/8) via ScalarE straight out of PSUM, bf16 into SBUF
  O'^T = V_aug^T P^T  with V augmented by a ones column -> row 64 = softmax Z

Scheduling: projection matmul chunks are interleaved into the attention
kt-loop (keeps the PE dense, HAM warm, and hides the projection phase under
ScalarE exp work).  The AllToAll is split in two: token windows are
b-interleaved (window j = b0[256j:256j+256] + b1[256j:256j+256]) so the first
A2A + half the W_O run under batch-1 attention.
"""

import os
import sys

for _p in ("/opt/trn_rl_repo",):
    if os.path.isdir(_p) and _p not in sys.path:
        sys.path.insert(0, _p)

from contextlib import ExitStack

import ml_dtypes
import numpy as np

import concourse.bass as bass
import concourse.tile as tile
from concourse import bacc, mybir
from concourse.bass import ts
from concourse.bass_utils import run_bass_kernel_spmd

BF16 = mybir.dt.bfloat16
F32 = mybir.dt.float32
AF = mybir.ActivationFunctionType

B, S, D = 2, 2048, 1024
H, HD = 16, 64
SC = 2048              # cache length
KT_N = (SC + S) // 128  # 32 key tiles of 128
NCORES = 8
T = B * S              # 4096 tokens, flat b-major
TPC = T // NCORES      # 512 tokens per core
HTPC = TPC // 2        # 256-token half windows
SCALE = 1.0 / np.sqrt(np.float32(HD))

bf = ml_dtypes.bfloat16


def _build_body(ctx: ExitStack, tc, io):
    nc = tc.nc
    xT, wq, wk, wv, wo, kTc, vc = (
        io["xT"], io["wq"], io["wk"], io["wv"], io["wo"], io["kTc"], io["vc"],
    )
    kT_new, v_new, out_shard = io["kT_new"], io["v_new"], io["out_shard"]

    sb = ctx.enter_context(tc.tile_pool(name="sb", bufs=1))
    dramp = ctx.enter_context(tc.tile_pool(name="dramp", bufs=1, space="DRAM"))

    # ---------------- input loads (spread across DMA queues) ----------------
    wq_sb = sb.tile([128, 8, 128], BF16)
    wk_sb = sb.tile([128, 8, 128], BF16)
    wv_sb = sb.tile([128, 8, 128], BF16)
    for kt in range(8):
        nc.gpsimd.dma_start(out=wq_sb[:, kt, :], in_=wq[ts(kt, 128), :])

    xt_sb = sb.tile([128, 8, T], BF16)
    for kt in range(8):
        eng = nc.sync if kt % 2 == 0 else nc.scalar
        eng.dma_start(out=xt_sb[:, kt, :], in_=xT[ts(kt, 128), :])

    kTc_sb = sb.tile([128, B, SC], BF16)
    for b in range(B):
        nc.sync.dma_start(out=kTc_sb[:, b, :], in_=kTc[b])

    # V augmented with a ones column (col 64 of each 65-wide block):
    # memset everything to 1.0, then overwrite cols 0:64 per key tile.
    vaug = []  # per local head: [128, B, 32, 65]
    for h in range(2):
        va = sb.tile([128, B, KT_N, 65], BF16, name=f"vaug{h}")
        nc.vector.memset(va, 1.0)
        for b in range(B):
            nc.sync.dma_start(
                out=va[:, b, 0:16, 0:64],
                in_=vc[b, h].rearrange("(j p) d -> p j d", p=128),
            )
        vaug.append(va)

    for kt in range(8):
        nc.gpsimd.dma_start(out=wk_sb[:, kt, :], in_=wk[ts(kt, 128), :])
        nc.gpsimd.dma_start(out=wv_sb[:, kt, :], in_=wv[ts(kt, 128), :])

    wo_sb = sb.tile([128, 8, D], BF16)
    for kt in range(8):
        nc.gpsimd.dma_start(out=wo_sb[:, kt, :], in_=wo[ts(kt, 128), :])

    # ---------------- shared PSUM pools ----------------
    # "mm" slots ([128,1024] = 2 banks) are shared by scores tiles and all
    # projection / W_O accumulators.  op0/op1 hold the per-head O' + Z rows.
    mm_ps = ctx.enter_context(tc.tile_pool(name="mm_ps", bufs=2, space="PSUM"))
    op_ps = ctx.enter_context(tc.tile_pool(name="op_ps", bufs=2, space="PSUM"))
    ptp = ctx.enter_context(tc.tile_pool(name="ptp", bufs=3))
    nrm = ctx.enter_context(tc.tile_pool(name="nrm", bufs=4))
    obp = ctx.enter_context(tc.tile_pool(name="obp", bufs=4))

    qt_sb = sb.tile([128, T], BF16)
    ktn_sb = sb.tile([128, T], BF16)
    ot_sb = sb.tile([128, T], BF16)  # normalized O^T, 2 heads stacked

    # ---------------- projection chunk emitters ----------------
    def q_chunk(nt):
        acc = mm_ps.tile([128, 512], F32, tag="mm", name=f"qacc{nt}")
        for kt in range(8):
            nc.tensor.matmul(
                acc, lhsT=wq_sb[:, kt, :], rhs=xt_sb[:, kt, ts(nt, 512)],
                start=(kt == 0), stop=(kt == 7),
            )
        nc.vector.tensor_copy(qt_sb[:, ts(nt, 512)], acc)

    def k_chunk(nt):
        acc = mm_ps.tile([128, 512], F32, tag="mm", name=f"kacc{nt}")
        for kt in range(8):
            nc.tensor.matmul(
                acc, lhsT=wk_sb[:, kt, :], rhs=xt_sb[:, kt, ts(nt, 512)],
                start=(kt == 0), stop=(kt == 7),
            )
        nc.vector.tensor_copy(ktn_sb[:, ts(nt, 512)], acc)
        if nt == 7:
            nc.sync.dma_start(out=kT_new[:, :], in_=ktn_sb)

    def v_chunk(mt):
        accv = mm_ps.tile([128, 128], F32, tag="mm", name=f"vacc{mt}")
        for kt in range(8):
            nc.tensor.matmul(
                accv, lhsT=xt_sb[:, kt, ts(mt, 128)], rhs=wv_sb[:, kt, :],
                start=(kt == 0), stop=(kt == 7),
            )
        b, j = divmod(mt, 16)
        nc.vector.tensor_copy(vaug[0][:, b, 16 + j, 0:64], accv[:, 0:64])
        nc.vector.tensor_copy(vaug[1][:, b, 16 + j, 0:64], accv[:, 64:128])
        if mt % 16 == 15:
            for h in range(2):
                nc.sync.dma_start(
                    out=v_new[b * S:(b + 1) * S, ts(h, 64)].rearrange(
                        "(j p) d -> p j d", p=128
                    ),
                    in_=vaug[h][:, b, 16:32, 0:64],
                )

    # feeder: proj chunks consumed between attention kt groups
    pending = []

    def feed(n=1):
        for _ in range(n):
            if pending:
                pending.pop(0)()

    # ---------------- attention chunk ----------------
    def attn_chunk(b, qt, kts):
        qlo = b * S + qt * 512
        if kts[0] == 0:
            op = [
                op_ps.tile([65, 512], F32, name=f"op{h}_{b}_{qt}", tag=f"op{h}")
                for h in range(2)
            ]
            attn_chunk.op[(b, qt)] = op
        op = attn_chunk.op[(b, qt)]
        for kt in kts:
            if kt < 16:
                ktile = kTc_sb[:, b, ts(kt, 128)]
            else:
                ktile = ktn_sb[:, b * S + (kt - 16) * 128:
                               b * S + (kt - 15) * 128]
            sc = mm_ps.tile([128, 1024], F32, tag="mm", name=f"sc_{b}_{qt}_{kt}")
            for h in range(2):
                nc.tensor.matmul(
                    sc[:, ts(h, 512)],
                    lhsT=ktile[ts(h, 64), :],
                    rhs=qt_sb[ts(h, 64), qlo:qlo + 512],
                    start=True, stop=True,
                )
            pt = ptp.tile([128, 1024], BF16, tag="pt")
            nc.scalar.activation(pt, sc, AF.Exp, scale=float(SCALE))
            for h in range(2):
                nc.tensor.matmul(
                    attn_chunk.op[(b, qt)][h],
                    lhsT=vaug[h][:, b, kt, :],
                    rhs=pt[:, ts(h, 512)],
                    start=(kt == 0), stop=(kt == KT_N - 1),
                )
            feed()
        if kts[-1] == KT_N - 1:
            # normalize: rows 0:64 of op[h] are O'; row 64 is Z
            for h in range(2):
                oph = attn_chunk.op[(b, qt)][h]
                rz = nrm.tile([1, 512], F32, tag="rz")
                nc.vector.reciprocal(rz, oph[64:65, :])
                rzd = dramp.tile([1, 512], F32, tag="rzd", bufs=4)
                nc.sync.dma_start(out=rzd, in_=rz)
                rzb = nrm.tile([64, 512], F32, tag="rzb")
                nc.sync.dma_start(
                    out=rzb,
                    in_=bass.AP(
                        tensor=rzd.tensor,
                        offset=rzd.offset,
                        ap=[[0, 64]] + [list(a) for a in rzd.ap[-1:]],
                    ),
                )
                nc.vector.tensor_mul(
                    ot_sb[ts(h, 64), qlo:qlo + 512], oph[0:64, :], rzb
                )

    attn_chunk.op = {}

    # ---------------- A2A halves + W_O halves ----------------
    cc_in = [dramp.tile([NCORES, 128, HTPC], BF16, name=f"cc_in{half}")
             for half in range(2)]
    cc_out = [dramp.tile([NCORES, 128, HTPC], BF16, name=f"cc_out{half}")
              for half in range(2)]
    otf = [sb.tile([128, NCORES, HTPC], BF16, name=f"otf{half}")
           for half in range(2)]

    def a2a_half(half):
        # window j, half b: ot_sb columns [b*S + 256j, +256)
        for j in range(NCORES):
            nc.sync.dma_start(
                out=cc_in[half][j],
                in_=ot_sb[:, half * S + j * HTPC: half * S + (j + 1) * HTPC],
            )
        nc.gpsimd.collective_compute(
            "AllToAll",
            mybir.AluOpType.bypass,
            replica_groups=[list(range(NCORES))],
            ins=[cc_in[half].opt()],
            outs=[cc_out[half].opt()],
        )
        for s in range(NCORES):
            # gpsimd queue: naturally ordered after its collective_compute
            nc.gpsimd.dma_start(out=otf[half][:, s, :], in_=cc_out[half][s])

    def wo_half(half):
        # out_shard rows [half*256, half*256+256) = tokens of batch `half`
        for mt in range(2):
            acc = mm_ps.tile([128, 1024], F32, tag="mm", name=f"wacc{half}{mt}")
            for nt in range(2):
                for kt in range(8):
                    nc.tensor.matmul(
                        acc[:, ts(nt, 512)],
                        lhsT=otf[half][:, kt, ts(mt, 128)],
                        rhs=wo_sb[:, kt, ts(nt, 512)],
                        start=(kt == 0), stop=(kt == 7),
                    )
            ob = obp.tile([128, 1024], F32, tag="ob")
            nc.vector.tensor_copy(ob, acc)
            nc.sync.dma_start(
                out=out_shard[half * TPC // 2 + mt * 128:
                              half * TPC // 2 + (mt + 1) * 128, :],
                in_=ob,
            )

    # ---------------- emission schedule ----------------
    q_chunk(0)

    # b0/qt0 cache kts while projections stream through `feed`
    pending = (
        [lambda nt=nt: q_chunk(nt) for nt in range(1, 4)]
        + [lambda nt=nt: k_chunk(nt) for nt in range(0, 4)]
        + [lambda mt=mt: v_chunk(mt) for mt in range(0, 16)]
    )
    attn_chunk(0, 0, list(range(0, 16)))
    # force any leftover b0 projections before the new-key half
    while pending:
        feed()
    pending = (
        [lambda nt=nt: q_chunk(nt) for nt in range(4, 8)]
        + [lambda nt=nt: k_chunk(nt) for nt in range(4, 8)]
        + [lambda mt=mt: v_chunk(mt) for mt in range(16, 32)]
    )
    attn_chunk(0, 0, list(range(16, KT_N)))
    for qt in range(1, 4):
        attn_chunk(0, qt, list(range(KT_N)))
    while pending:
        feed()

    a2a_half(0)  # overlaps batch-1 attention

    attn_chunk(1, 0, list(range(KT_N)))
    attn_chunk(1, 1, list(range(KT_N)))
    wo_half(0)  # A2A#1 has completed under the two chunks above
    attn_chunk(1, 2, list(range(KT_N)))
    attn_chunk(1, 3, list(range(KT_N)))

    a2a_half(1)
    wo_half(1)


def build_nc():
    nc = bacc.Bacc(
        "TRN2",
        target_bir_lowering=False,
        debug=False,
        num_devices=NCORES,
    )
    io = {
        "xT": nc.dram_tensor("xT", [D, T], BF16, kind="ExternalInput").ap(),
        "wq": nc.dram_tensor("wq", [D, 128], BF16, kind="ExternalInput").ap(),
        "wk": nc.dram_tensor("wk", [D, 128], BF16, kind="ExternalInput").ap(),
        "wv": nc.dram_tensor("wv", [D, 128], BF16, kind="ExternalInput").ap(),
        "wo": nc.dram_tensor("wo", [D, D], BF16, kind="ExternalInput").ap(),
        "kTc": nc.dram_tensor("kTc", [B, 128, SC], BF16, kind="ExternalInput").ap(),
        "vc": nc.dram_tensor("vc", [B, 2, SC, HD], BF16, kind="ExternalInput").ap(),
        "kT_new": nc.dram_tensor("kT_new", [128, T], BF16, kind="ExternalOutput").ap(),
        "v_new": nc.dram_tensor("v_new", [T, 128], BF16, kind="ExternalOutput").ap(),
        "out_shard": nc.dram_tensor(
            "out_shard", [TPC, D], F32, kind="ExternalOutput"
        ).ap(),
    }
    with tile.TileContext(nc) as tc, ExitStack() as ctx:
        _build_body(ctx, tc, io)
    nc.compile()
    return nc


def make_in_maps(x, kv_k, kv_v, W_Q, W_K, W_V, W_O):
    """Host-side shard/pre-transpose. All fp32 numpy in, bf16 shards out."""
    xT = np.ascontiguousarray(x.reshape(T, D).T).astype(bf)
    wo = W_O.astype(bf)
    in_maps = []
    for i in range(NCORES):
        hsl = slice(2 * i, 2 * i + 2)
        csl = slice(128 * i, 128 * (i + 1))
        # K cache, transposed to [B, 2*64, SC] with heads stacked on axis 1
        kTc = (
            kv_k[:, hsl]                      # [B, 2, SC, HD]
            .transpose(0, 1, 3, 2)            # [B, 2, HD, SC]
            .reshape(B, 128, SC)
            .astype(bf)
        )
        in_maps.append(
            {
                "xT": xT,
                "wq": np.ascontiguousarray(W_Q[:, csl]).astype(bf),
                "wk": np.ascontiguousarray(W_K[:, csl]).astype(bf),
                "wv": np.ascontiguousarray(W_V[:, csl]).astype(bf),
                "wo": wo,
                "kTc": np.ascontiguousarray(kTc),
                "vc": np.ascontiguousarray(kv_v[:, hsl]).astype(bf),
            }
        )
    return in_maps


def assemble(results, kv_k, kv_v):
    """Host-side unshard: rebuild (out, (K, V)) in fp32."""
    out = np.empty((T, D), np.float32)
    for i in range(NCORES):
        sh = results[i]["out_shard"]
        # rows 0:256 = b0 tokens [256i, 256i+256); rows 256:512 = b1 same
        out[i * HTPC:(i + 1) * HTPC] = sh[:HTPC]
        out[S + i * HTPC: S + (i + 1) * HTPC] = sh[HTPC:]
    out = out.reshape(B, S, D)

    k_new = np.empty((B, H, S, HD), np.float32)
    v_new = np.empty((B, H, S, HD), np.float32)
    for i in range(NCORES):
        # kT_new [128, T]: row p = h_local*64 + d, col t = b*S + s
        ktn = np.asarray(results[i]["kT_new"], dtype=np.float32).reshape(
            2, HD, B, S
        )  # [h_local, d, b, s]
        k_new[:, 2 * i:2 * i + 2] = ktn.transpose(2, 0, 3, 1)
        # v_new [T, 128]: row t = b*S + s, col c = h_local*64 + d
        vnn = np.asarray(results[i]["v_new"], dtype=np.float32).reshape(
            B, S, 2, HD
        )
        v_new[:, 2 * i:2 * i + 2] = vnn.transpose(0, 2, 1, 3)

    K = np.concatenate([np.asarray(kv_k, np.float32), k_new], axis=2)
    V = np.concatenate([np.asarray(kv_v, np.float32), v_new], axis=2)
    return out, (K, V)


_NC_CACHE = {}


def get_nc():
    if "nc" not in _NC_CACHE:
        _NC_CACHE["nc"] = build_nc()
    return _NC_CACHE["nc"]


def kernel(x, kv_k, kv_v, W_Q, W_K, W_V, W_O):
    x = np.asarray(x, np.float32)
    kv_k = np.asarray(kv_k, np.float32)
    kv_v = np.asarray(kv_v, np.float32)
    nc = get_nc()
    in_maps = make_in_maps(
        x, kv_k, kv_v,
        np.asarray(W_Q, np.float32), np.asarray(W_K, np.float32),
        np.asarray(W_V, np.float32), np.asarray(W_O, np.float32),
    )
    res = run_bass_kernel_spmd(nc, in_maps, core_ids=list(range(NCORES)))
    return assemble(res.results, kv_k, kv_v)


# revision 13
# speedup vs baseline: 1.0080x; 1.0080x over previous
"""Multi-head attention w/ KV cache, tensor-parallel over 8 TRN2 NeuronCores.

Sharding: heads are split 2-per-core (W_Q/W_K/W_V column shards, KV cache head
shards).  Each core computes Q/K/V projections for its 2 heads, full attention
over the 4096-key axis (2048 cache + 2048 new), then two AllToAlls convert the
head-sharded attention output O^T into a token-sharded full-depth O^T so every
core computes W_O for its own 512-token shard — no all-reduce and no
core-dependent control flow (pure SPMD).

Layout notes (everything transposed so the PE contracts over partitions):
  x^T        [1024, 4096]  d_model on partitions, tokens b-major on free axis
  Q^T/K^T    [128, 4096]   2 heads stacked on partitions (64 rows each)
  scores^T   [keys, q]     K^T stationary, Q^T moving, 2 heads row-packed
  P = exp(s/8) via ScalarE straight out of PSUM, bf16 into SBUF
  O'^T = V_aug^T P^T  with V augmented by a ones column -> row 64 = softmax Z

Scheduling: projection matmul chunks are interleaved into the attention
kt-loop (keeps the PE dense, HAM warm, and hides the projection phase under
ScalarE exp work).  The AllToAll is split in two: token windows are
b-interleaved (window j = b0[256j:256j+256] + b1[256j:256j+256]) so the first
A2A + half the W_O run under batch-1 attention.
"""

import os
import sys

for _p in ("/opt/trn_rl_repo",):
    if os.path.isdir(_p) and _p not in sys.path:
        sys.path.insert(0, _p)

from contextlib import ExitStack

import ml_dtypes
import numpy as np

import concourse.bass as bass
import concourse.tile as tile
from concourse import bacc, mybir
from concourse.bass import ts
from concourse.bass_utils import run_bass_kernel_spmd

BF16 = mybir.dt.bfloat16
F32 = mybir.dt.float32
AF = mybir.ActivationFunctionType

B, S, D = 2, 2048, 1024
H, HD = 16, 64
SC = 2048              # cache length
KT_N = (SC + S) // 128  # 32 key tiles of 128
NCORES = 8
T = B * S              # 4096 tokens, flat b-major
TPC = T // NCORES      # 512 tokens per core
HTPC = TPC // 2        # 256-token half windows
SCALE = 1.0 / np.sqrt(np.float32(HD))

bf = ml_dtypes.bfloat16


def _build_body(ctx: ExitStack, tc, io):
    nc = tc.nc
    xT, wq, wk, wv, wo, kTc, vc = (
        io["xT"], io["wq"], io["wk"], io["wv"], io["wo"], io["kTc"], io["vc"],
    )
    kT_new, v_new, out_shard = io["kT_new"], io["v_new"], io["out_shard"]

    sb = ctx.enter_context(tc.tile_pool(name="sb", bufs=1))
    dramp = ctx.enter_context(tc.tile_pool(name="dramp", bufs=1, space="DRAM"))

    # ---------------- input loads (spread across DMA queues) ----------------
    wq_sb = sb.tile([128, 8, 128], BF16)
    wk_sb = sb.tile([128, 8, 128], BF16)
    wv_sb = sb.tile([128, 8, 128], BF16)
    for kt in range(8):
        nc.gpsimd.dma_start(out=wq_sb[:, kt, :], in_=wq[ts(kt, 128), :])

    xt_sb = sb.tile([128, 8, T], BF16)
    for kt in range(8):
        eng = nc.sync if kt % 2 == 0 else nc.scalar
        eng.dma_start(out=xt_sb[:, kt, :], in_=xT[ts(kt, 128), :])

    kTc_sb = sb.tile([128, B, SC], BF16)
    for b in range(B):
        nc.sync.dma_start(out=kTc_sb[:, b, :], in_=kTc[b])

    # V augmented with a ones column (col 64 of each 65-wide block):
    # memset everything to 1.0, then overwrite cols 0:64 per key tile.
    vaug = []  # per local head: [128, B, 32, 65]
    for h in range(2):
        va = sb.tile([128, B, KT_N, 65], BF16, name=f"vaug{h}")
        nc.vector.memset(va, 1.0)
        for b in range(B):
            nc.sync.dma_start(
                out=va[:, b, 0:16, 0:64],
                in_=vc[b, h].rearrange("(j p) d -> p j d", p=128),
            )
        vaug.append(va)

    for kt in range(8):
        nc.gpsimd.dma_start(out=wk_sb[:, kt, :], in_=wk[ts(kt, 128), :])
        nc.gpsimd.dma_start(out=wv_sb[:, kt, :], in_=wv[ts(kt, 128), :])

    wo_sb = sb.tile([128, 8, D], BF16)
    for kt in range(8):
        nc.gpsimd.dma_start(out=wo_sb[:, kt, :], in_=wo[ts(kt, 128), :])

    # ---------------- PSUM pools (8 banks total, exact fit) ----------------
    # scores: 2 x [128,1024] = 4 banks; proj accumulators: 2 x [128,512] =
    # 2 banks; op0/op1 (per-head O' + Z row, [65,512]) 1 bank each.
    sc_ps = ctx.enter_context(tc.tile_pool(name="sc_ps", bufs=2, space="PSUM"))
    pr_ps = ctx.enter_context(tc.tile_pool(name="pr_ps", bufs=2, space="PSUM"))
    op_ps = ctx.enter_context(tc.tile_pool(name="op_ps", bufs=1, space="PSUM"))
    ptp = ctx.enter_context(tc.tile_pool(name="ptp", bufs=3))
    nrm = ctx.enter_context(tc.tile_pool(name="nrm", bufs=4))
    obp = ctx.enter_context(tc.tile_pool(name="obp", bufs=4))

    qt_sb = sb.tile([128, T], BF16)
    ktn_sb = sb.tile([128, T], BF16)
    ot_sb = sb.tile([128, T], BF16)  # normalized O^T, 2 heads stacked

    # ---------------- projection chunk emitters ----------------
    def q_chunk(nt):
        acc = pr_ps.tile([128, 512], F32, tag="pr", name=f"qacc{nt}")
        for kt in range(8):
            nc.tensor.matmul(
                acc, lhsT=wq_sb[:, kt, :], rhs=xt_sb[:, kt, ts(nt, 512)],
                start=(kt == 0), stop=(kt == 7),
            )
        nc.vector.tensor_copy(qt_sb[:, ts(nt, 512)], acc)

    def k_chunk(nt):
        acc = pr_ps.tile([128, 512], F32, tag="pr", name=f"kacc{nt}")
        for kt in range(8):
            nc.tensor.matmul(
                acc, lhsT=wk_sb[:, kt, :], rhs=xt_sb[:, kt, ts(nt, 512)],
                start=(kt == 0), stop=(kt == 7),
            )
        nc.vector.tensor_copy(ktn_sb[:, ts(nt, 512)], acc)
        if nt == 7:
            nc.sync.dma_start(out=kT_new[:, :], in_=ktn_sb)

    def v_chunk(mt):
        accv = pr_ps.tile([128, 128], F32, tag="pr", name=f"vacc{mt}")
        for kt in range(8):
            nc.tensor.matmul(
                accv, lhsT=xt_sb[:, kt, ts(mt, 128)], rhs=wv_sb[:, kt, :],
                start=(kt == 0), stop=(kt == 7),
            )
        b, j = divmod(mt, 16)
        nc.vector.tensor_copy(vaug[0][:, b, 16 + j, 0:64], accv[:, 0:64])
        nc.vector.tensor_copy(vaug[1][:, b, 16 + j, 0:64], accv[:, 64:128])
        if mt % 16 == 15:
            for h in range(2):
                nc.sync.dma_start(
                    out=v_new[b * S:(b + 1) * S, ts(h, 64)].rearrange(
                        "(j p) d -> p j d", p=128
                    ),
                    in_=vaug[h][:, b, 16:32, 0:64],
                )

    # feeder: proj chunks consumed between attention kt groups
    pending = []

    def feed(n=1):
        for _ in range(n):
            if pending:
                pending.pop(0)()

    # ---------------- attention chunk ----------------
    def attn_chunk(b, qt, kts):
        qlo = b * S + qt * 512
        if kts[0] == 0:
            op = [
                op_ps.tile([65, 512], F32, name=f"op{h}_{b}_{qt}", tag=f"op{h}")
                for h in range(2)
            ]
            attn_chunk.op[(b, qt)] = op
        op = attn_chunk.op[(b, qt)]
        for kt in kts:
            if kt < 16:
                ktile = kTc_sb[:, b, ts(kt, 128)]
            else:
                ktile = ktn_sb[:, b * S + (kt - 16) * 128:
                               b * S + (kt - 15) * 128]
            sc = sc_ps.tile([128, 1024], F32, tag="sc", name=f"sc_{b}_{qt}_{kt}")
            for h in range(2):
                nc.tensor.matmul(
                    sc[:, ts(h, 512)],
                    lhsT=ktile[ts(h, 64), :],
                    rhs=qt_sb[ts(h, 64), qlo:qlo + 512],
                    start=True, stop=True,
                )
            pt = ptp.tile([128, 1024], BF16, tag="pt")
            nc.scalar.activation(pt, sc, AF.Exp, scale=float(SCALE))
            for h in range(2):
                nc.tensor.matmul(
                    attn_chunk.op[(b, qt)][h],
                    lhsT=vaug[h][:, b, kt, :],
                    rhs=pt[:, ts(h, 512)],
                    start=(kt == 0), stop=(kt == KT_N - 1),
                )
            feed()
        if kts[-1] == KT_N - 1:
            # normalize: rows 0:64 of op[h] are O'; row 64 is Z
            for h in range(2):
                oph = attn_chunk.op[(b, qt)][h]
                rz = nrm.tile([1, 512], F32, tag="rz")
                nc.vector.reciprocal(rz, oph[64:65, :])
                rzd = dramp.tile([1, 512], F32, tag="rzd", bufs=4)
                nc.sync.dma_start(out=rzd, in_=rz)
                rzb = nrm.tile([64, 512], F32, tag="rzb")
                nc.sync.dma_start(
                    out=rzb,
                    in_=bass.AP(
                        tensor=rzd.tensor,
                        offset=rzd.offset,
                        ap=[[0, 64]] + [list(a) for a in rzd.ap[-1:]],
                    ),
                )
                nc.vector.tensor_mul(
                    ot_sb[ts(h, 64), qlo:qlo + 512], oph[0:64, :], rzb
                )

    attn_chunk.op = {}

    # ---------------- A2A halves + W_O halves ----------------
    cc_in = [dramp.tile([NCORES, 128, HTPC], BF16, name=f"cc_in{half}")
             for half in range(2)]
    cc_out = [dramp.tile([NCORES, 128, HTPC], BF16, name=f"cc_out{half}")
              for half in range(2)]
    otf = [sb.tile([128, NCORES, HTPC], BF16, name=f"otf{half}")
           for half in range(2)]

    def a2a_half(half):
        # window j, half b: ot_sb columns [b*S + 256j, +256)
        for j in range(NCORES):
            nc.sync.dma_start(
                out=cc_in[half][j],
                in_=ot_sb[:, half * S + j * HTPC: half * S + (j + 1) * HTPC],
            )
        nc.gpsimd.collective_compute(
            "AllToAll",
            mybir.AluOpType.bypass,
            replica_groups=[list(range(NCORES))],
            ins=[cc_in[half].opt()],
            outs=[cc_out[half].opt()],
        )
        for s in range(NCORES):
            # gpsimd queue: naturally ordered after its collective_compute
            nc.gpsimd.dma_start(out=otf[half][:, s, :], in_=cc_out[half][s])

    def wo_half(half):
        # out_shard rows [half*256, half*256+256) = tokens of batch `half`
        for mt in range(2):
            acc = sc_ps.tile([128, 1024], F32, tag="sc", name=f"wacc{half}{mt}")
            for nt in range(2):
                for kt in range(8):
                    nc.tensor.matmul(
                        acc[:, ts(nt, 512)],
                        lhsT=otf[half][:, kt, ts(mt, 128)],
                        rhs=wo_sb[:, kt, ts(nt, 512)],
                        start=(kt == 0), stop=(kt == 7),
                    )
            ob = obp.tile([128, 1024], F32, tag="ob")
            nc.vector.tensor_copy(ob, acc)
            nc.sync.dma_start(
                out=out_shard[half * TPC // 2 + mt * 128:
                              half * TPC // 2 + (mt + 1) * 128, :],
                in_=ob,
            )

    # ---------------- emission schedule ----------------
    q_chunk(0)

    # b0/qt0 cache kts while projections stream through `feed`
    pending = (
        [lambda nt=nt: q_chunk(nt) for nt in range(1, 4)]
        + [lambda nt=nt: k_chunk(nt) for nt in range(0, 4)]
        + [lambda mt=mt: v_chunk(mt) for mt in range(0, 16)]
    )
    attn_chunk(0, 0, list(range(0, 16)))
    # force any leftover b0 projections before the new-key half
    while pending:
        feed()
    pending = (
        [lambda nt=nt: q_chunk(nt) for nt in range(4, 8)]
        + [lambda nt=nt: k_chunk(nt) for nt in range(4, 8)]
        + [lambda mt=mt: v_chunk(mt) for mt in range(16, 32)]
    )
    attn_chunk(0, 0, list(range(16, KT_N)))
    for qt in range(1, 4):
        attn_chunk(0, qt, list(range(KT_N)))
    while pending:
        feed()

    a2a_half(0)  # overlaps batch-1 attention

    attn_chunk(1, 0, list(range(KT_N)))
    attn_chunk(1, 1, list(range(KT_N)))
    wo_half(0)  # A2A#1 has completed under the two chunks above
    attn_chunk(1, 2, list(range(KT_N)))
    attn_chunk(1, 3, list(range(KT_N)))

    a2a_half(1)
    wo_half(1)


def build_nc():
    nc = bacc.Bacc(
        "TRN2",
        target_bir_lowering=False,
        debug=False,
        num_devices=NCORES,
    )
    io = {
        "xT": nc.dram_tensor("xT", [D, T], BF16, kind="ExternalInput").ap(),
        "wq": nc.dram_tensor("wq", [D, 128], BF16, kind="ExternalInput").ap(),
        "wk": nc.dram_tensor("wk", [D, 128], BF16, kind="ExternalInput").ap(),
        "wv": nc.dram_tensor("wv", [D, 128], BF16, kind="ExternalInput").ap(),
        "wo": nc.dram_tensor("wo", [D, D], BF16, kind="ExternalInput").ap(),
        "kTc": nc.dram_tensor("kTc", [B, 128, SC], BF16, kind="ExternalInput").ap(),
        "vc": nc.dram_tensor("vc", [B, 2, SC, HD], BF16, kind="ExternalInput").ap(),
        "kT_new": nc.dram_tensor("kT_new", [128, T], BF16, kind="ExternalOutput").ap(),
        "v_new": nc.dram_tensor("v_new", [T, 128], BF16, kind="ExternalOutput").ap(),
        "out_shard": nc.dram_tensor(
            "out_shard", [TPC, D], F32, kind="ExternalOutput"
        ).ap(),
    }
    with tile.TileContext(nc) as tc, ExitStack() as ctx:
        _build_body(ctx, tc, io)
    nc.compile()
    return nc


def make_in_maps(x, kv_k, kv_v, W_Q, W_K, W_V, W_O):
    """Host-side shard/pre-transpose. All fp32 numpy in, bf16 shards out."""
    xT = np.ascontiguousarray(x.reshape(T, D).T).astype(bf)
    wo = W_O.astype(bf)
    in_maps = []
    for i in range(NCORES):
        hsl = slice(2 * i, 2 * i + 2)
        csl = slice(128 * i, 128 * (i + 1))
        # K cache, transposed to [B, 2*64, SC] with heads stacked on axis 1
        kTc = (
            kv_k[:, hsl]                      # [B, 2, SC, HD]
            .transpose(0, 1, 3, 2)            # [B, 2, HD, SC]
            .reshape(B, 128, SC)
            .astype(bf)
        )
        in_maps.append(
            {
                "xT": xT,
                "wq": np.ascontiguousarray(W_Q[:, csl]).astype(bf),
                "wk": np.ascontiguousarray(W_K[:, csl]).astype(bf),
                "wv": np.ascontiguousarray(W_V[:, csl]).astype(bf),
                "wo": wo,
                "kTc": np.ascontiguousarray(kTc),
                "vc": np.ascontiguousarray(kv_v[:, hsl]).astype(bf),
            }
        )
    return in_maps


def assemble(results, kv_k, kv_v):
    """Host-side unshard: rebuild (out, (K, V)) in fp32."""
    out = np.empty((T, D), np.float32)
    for i in range(NCORES):
        sh = results[i]["out_shard"]
        # rows 0:256 = b0 tokens [256i, 256i+256); rows 256:512 = b1 same
        out[i * HTPC:(i + 1) * HTPC] = sh[:HTPC]
        out[S + i * HTPC: S + (i + 1) * HTPC] = sh[HTPC:]
    out = out.reshape(B, S, D)

    k_new = np.empty((B, H, S, HD), np.float32)
    v_new = np.empty((B, H, S, HD), np.float32)
    for i in range(NCORES):
        # kT_new [128, T]: row p = h_local*64 + d, col t = b*S + s
        ktn = np.asarray(results[i]["kT_new"], dtype=np.float32).reshape(
            2, HD, B, S
        )  # [h_local, d, b, s]
        k_new[:, 2 * i:2 * i + 2] = ktn.transpose(2, 0, 3, 1)
        # v_new [T, 128]: row t = b*S + s, col c = h_local*64 + d
        vnn = np.asarray(results[i]["v_new"], dtype=np.float32).reshape(
            B, S, 2, HD
        )
        v_new[:, 2 * i:2 * i + 2] = vnn.transpose(0, 2, 1, 3)

    K = np.concatenate([np.asarray(kv_k, np.float32), k_new], axis=2)
    V = np.concatenate([np.asarray(kv_v, np.float32), v_new], axis=2)
    return out, (K, V)


_NC_CACHE = {}


def get_nc():
    if "nc" not in _NC_CACHE:
        _NC_CACHE["nc"] = build_nc()
    return _NC_CACHE["nc"]


def kernel(x, kv_k, kv_v, W_Q, W_K, W_V, W_O):
    x = np.asarray(x, np.float32)
    kv_k = np.asarray(kv_k, np.float32)
    kv_v = np.asarray(kv_v, np.float32)
    nc = get_nc()
    in_maps = make_in_maps(
        x, kv_k, kv_v,
        np.asarray(W_Q, np.float32), np.asarray(W_K, np.float32),
        np.asarray(W_V, np.float32), np.asarray(W_O, np.float32),
    )
    res = run_bass_kernel_spmd(nc, in_maps, core_ids=list(range(NCORES)))
    return assemble(res.results, kv_k, kv_v)


# revision 15
# speedup vs baseline: 1.0672x; 1.0587x over previous
"""Multi-head attention w/ KV cache, tensor-parallel over 8 TRN2 NeuronCores.

Sharding: heads are split 2-per-core (W_Q/W_K/W_V column shards, KV cache head
shards).  Each core computes Q/K/V projections for its 2 heads, full attention
over the 4096-key axis (2048 cache + 2048 new), then two AllToAlls convert the
head-sharded attention output O^T into a token-sharded full-depth O^T so every
core computes W_O for its own 512-token shard — no all-reduce and no
core-dependent control flow (pure SPMD).

Layout notes (everything transposed so the PE contracts over partitions):
  x^T        [1024, 4096]  d_model on partitions, tokens b-major on free axis
  Q^T/K^T    [128, 4096]   2 heads stacked on partitions (64 rows each)
  scores^T   [keys, q]     K^T stationary, Q^T moving, 2 heads row-packed
  P = exp(s/8) via ScalarE straight out of PSUM, bf16 into SBUF
  O'^T = V_aug^T P^T  with V augmented by a ones column -> row 64 = softmax Z

Scheduling: projection matmul chunks are interleaved into the attention
kt-loop (keeps the PE dense, HAM warm, and hides the projection phase under
ScalarE exp work).  The AllToAll is split in two: token windows are
b-interleaved (window j = b0[256j:256j+256] + b1[256j:256j+256]) so the first
A2A + half the W_O run under batch-1 attention.
"""

import os
import sys

for _p in ("/opt/trn_rl_repo",):
    if os.path.isdir(_p) and _p not in sys.path:
        sys.path.insert(0, _p)

from contextlib import ExitStack

import ml_dtypes
import numpy as np

import concourse.bass as bass
import concourse.tile as tile
from concourse import bacc, mybir
from concourse.bass import ts
from concourse.bass_utils import run_bass_kernel_spmd

BF16 = mybir.dt.bfloat16
F32 = mybir.dt.float32
AF = mybir.ActivationFunctionType

B, S, D = 2, 2048, 1024
H, HD = 16, 64
SC = 2048              # cache length
KT_N = (SC + S) // 128  # 32 key tiles of 128
NCORES = 8
T = B * S              # 4096 tokens, flat b-major
TPC = T // NCORES      # 512 tokens per core
HTPC = TPC // 2        # 256-token half windows
SCALE = 1.0 / np.sqrt(np.float32(HD))

bf = ml_dtypes.bfloat16


def _build_body(ctx: ExitStack, tc, io):
    nc = tc.nc
    xT, wq, wk, wv, wo, kTc, vc = (
        io["xT"], io["wq"], io["wk"], io["wv"], io["wo"], io["kTc"], io["vc"],
    )
    kT_new, v_new, out_shard = io["kT_new"], io["v_new"], io["out_shard"]

    sb = ctx.enter_context(tc.tile_pool(name="sb", bufs=1))
    dramp = ctx.enter_context(tc.tile_pool(name="dramp", bufs=1, space="DRAM"))

    # ---------------- input loads (spread across DMA queues) ----------------
    wq_sb = sb.tile([128, 8, 128], BF16)
    wk_sb = sb.tile([128, 8, 128], BF16)
    wv_sb = sb.tile([128, 8, 128], BF16)
    for kt in range(8):
        nc.gpsimd.dma_start(out=wq_sb[:, kt, :], in_=wq[ts(kt, 128), :])

    kTc_sb = sb.tile([128, B, SC], BF16)
    for b in range(B):
        nc.scalar.dma_start(out=kTc_sb[:, b, :], in_=kTc[b])

    # V augmented with a ones column (col 64 of each 65-wide block):
    # memset everything to 1.0, then overwrite cols 0:64 per key tile.
    vaug = []  # per local head: [128, B, 32, 65]
    for h in range(2):
        va = sb.tile([128, B, KT_N, 65], BF16, name=f"vaug{h}")
        nc.vector.memset(va, 1.0)
        for b in range(B):
            nc.scalar.dma_start(
                out=va[:, b, 0:16, 0:64],
                in_=vc[b, h].rearrange("(j p) d -> p j d", p=128),
            )
        vaug.append(va)

    xt_sb = sb.tile([128, 8, T], BF16)
    for kt in range(8):
        eng = nc.sync if kt % 2 == 0 else nc.scalar
        eng.dma_start(out=xt_sb[:, kt, :], in_=xT[ts(kt, 128), :])

    for kt in range(8):
        nc.gpsimd.dma_start(out=wk_sb[:, kt, :], in_=wk[ts(kt, 128), :])
        nc.gpsimd.dma_start(out=wv_sb[:, kt, :], in_=wv[ts(kt, 128), :])

    wo_sb = sb.tile([128, 8, D], BF16)
    for kt in range(8):
        nc.gpsimd.dma_start(out=wo_sb[:, kt, :], in_=wo[ts(kt, 128), :])

    # ---------------- PSUM pools (8 banks total, exact fit) ----------------
    # scores: 2 x [128,1024] = 4 banks; proj accumulators: 2 x [128,512] =
    # 2 banks; op0/op1 (per-head O' + Z row, [65,512]) 1 bank each.
    sc_ps = ctx.enter_context(tc.tile_pool(name="sc_ps", bufs=2, space="PSUM"))
    pr_ps = ctx.enter_context(tc.tile_pool(name="pr_ps", bufs=2, space="PSUM"))
    op_ps = ctx.enter_context(tc.tile_pool(name="op_ps", bufs=1, space="PSUM"))
    ptp = ctx.enter_context(tc.tile_pool(name="ptp", bufs=3))
    nrm = ctx.enter_context(tc.tile_pool(name="nrm", bufs=4))
    obp = ctx.enter_context(tc.tile_pool(name="obp", bufs=4))

    qt_sb = sb.tile([128, T], BF16)
    ktn_sb = sb.tile([128, T], BF16)
    ot_sb = sb.tile([128, T], BF16)  # normalized O^T, 2 heads stacked

    # ---------------- projection chunk emitters ----------------
    def q_chunk(nt):
        acc = pr_ps.tile([128, 512], F32, tag="pr", name=f"qacc{nt}")
        for kt in range(8):
            nc.tensor.matmul(
                acc, lhsT=wq_sb[:, kt, :], rhs=xt_sb[:, kt, ts(nt, 512)],
                start=(kt == 0), stop=(kt == 7),
            )
        nc.vector.tensor_copy(qt_sb[:, ts(nt, 512)], acc)

    def k_chunk(nt):
        acc = pr_ps.tile([128, 512], F32, tag="pr", name=f"kacc{nt}")
        for kt in range(8):
            nc.tensor.matmul(
                acc, lhsT=wk_sb[:, kt, :], rhs=xt_sb[:, kt, ts(nt, 512)],
                start=(kt == 0), stop=(kt == 7),
            )
        nc.vector.tensor_copy(ktn_sb[:, ts(nt, 512)], acc)
        if nt == 7:
            nc.sync.dma_start(out=kT_new[:, :], in_=ktn_sb)

    def v_chunk(mt):
        accv = pr_ps.tile([128, 128], F32, tag="pr", name=f"vacc{mt}")
        for kt in range(8):
            nc.tensor.matmul(
                accv, lhsT=xt_sb[:, kt, ts(mt, 128)], rhs=wv_sb[:, kt, :],
                start=(kt == 0), stop=(kt == 7),
            )
        b, j = divmod(mt, 16)
        nc.vector.tensor_copy(vaug[0][:, b, 16 + j, 0:64], accv[:, 0:64])
        nc.vector.tensor_copy(vaug[1][:, b, 16 + j, 0:64], accv[:, 64:128])
        if mt % 16 == 15:
            for h in range(2):
                nc.sync.dma_start(
                    out=v_new[b * S:(b + 1) * S, ts(h, 64)].rearrange(
                        "(j p) d -> p j d", p=128
                    ),
                    in_=vaug[h][:, b, 16:32, 0:64],
                )

    # feeder: proj chunks consumed between attention kt groups
    pending = []

    def feed(n=1):
        for _ in range(n):
            if pending:
                pending.pop(0)()

    # ---------------- attention chunk ----------------
    def attn_chunk(b, qt, kts):
        qlo = b * S + qt * 512
        if kts[0] == 0:
            op = [
                op_ps.tile([65, 512], F32, name=f"op{h}_{b}_{qt}", tag=f"op{h}")
                for h in range(2)
            ]
            attn_chunk.op[(b, qt)] = op
        op = attn_chunk.op[(b, qt)]
        for kt in kts:
            if kt < 16:
                ktile = kTc_sb[:, b, ts(kt, 128)]
            else:
                ktile = ktn_sb[:, b * S + (kt - 16) * 128:
                               b * S + (kt - 15) * 128]
            sc = sc_ps.tile([128, 1024], F32, tag="sc", name=f"sc_{b}_{qt}_{kt}")
            for h in range(2):
                nc.tensor.matmul(
                    sc[:, ts(h, 512)],
                    lhsT=ktile[ts(h, 64), :],
                    rhs=qt_sb[ts(h, 64), qlo:qlo + 512],
                    start=True, stop=True,
                )
            pt = ptp.tile([128, 1024], BF16, tag="pt")
            nc.scalar.activation(pt, sc, AF.Exp, scale=float(SCALE))
            for h in range(2):
                nc.tensor.matmul(
                    attn_chunk.op[(b, qt)][h],
                    lhsT=vaug[h][:, b, kt, :],
                    rhs=pt[:, ts(h, 512)],
                    start=(kt == 0), stop=(kt == KT_N - 1),
                )
            feed()
        if kts[-1] == KT_N - 1:
            # Evacuate O'+Z from PSUM fast (frees the op banks for the next
            # chunk), then normalize from SBUF off the PE critical path.
            for h in range(2):
                oph = attn_chunk.op[(b, qt)][h]
                orw = nrm.tile([65, 512], F32, tag="orw", bufs=3)
                nc.vector.tensor_copy(orw, oph)
                rz = nrm.tile([1, 512], F32, tag="rz")
                nc.vector.reciprocal(rz, orw[64:65, :])
                rzd = dramp.tile([1, 512], F32, tag="rzd", bufs=4)
                nc.sync.dma_start(out=rzd, in_=rz)
                rzb = nrm.tile([64, 512], F32, tag="rzb")
                nc.sync.dma_start(
                    out=rzb,
                    in_=bass.AP(
                        tensor=rzd.tensor,
                        offset=rzd.offset,
                        ap=[[0, 64]] + [list(a) for a in rzd.ap[-1:]],
                    ),
                )
                nc.vector.tensor_mul(
                    ot_sb[ts(h, 64), qlo:qlo + 512], orw[0:64, :], rzb
                )

    attn_chunk.op = {}

    # ---------------- A2A halves + W_O halves ----------------
    cc_in = [dramp.tile([NCORES, 128, HTPC], BF16, name=f"cc_in{half}")
             for half in range(2)]
    cc_out = [dramp.tile([NCORES, 128, HTPC], BF16, name=f"cc_out{half}")
              for half in range(2)]
    otf = [sb.tile([128, NCORES, HTPC], BF16, name=f"otf{half}")
           for half in range(2)]

    def a2a_half(half):
        # window j, half b: ot_sb columns [b*S + 256j, +256)
        for j in range(NCORES):
            nc.sync.dma_start(
                out=cc_in[half][j],
                in_=ot_sb[:, half * S + j * HTPC: half * S + (j + 1) * HTPC],
            )
        nc.gpsimd.collective_compute(
            "AllToAll",
            mybir.AluOpType.bypass,
            replica_groups=[list(range(NCORES))],
            ins=[cc_in[half].opt()],
            outs=[cc_out[half].opt()],
        )
        for s in range(NCORES):
            # gpsimd queue: naturally ordered after its collective_compute
            nc.gpsimd.dma_start(out=otf[half][:, s, :], in_=cc_out[half][s])

    def wo_half(half):
        # out_shard rows [half*256, half*256+256) = tokens of batch `half`
        for mt in range(2):
            acc = sc_ps.tile([128, 1024], F32, tag="sc", name=f"wacc{half}{mt}")
            for nt in range(2):
                for kt in range(8):
                    nc.tensor.matmul(
                        acc[:, ts(nt, 512)],
                        lhsT=otf[half][:, kt, ts(mt, 128)],
                        rhs=wo_sb[:, kt, ts(nt, 512)],
                        start=(kt == 0), stop=(kt == 7),
                    )
            ob = obp.tile([128, 1024], F32, tag="ob")
            nc.vector.tensor_copy(ob, acc)
            nc.sync.dma_start(
                out=out_shard[half * TPC // 2 + mt * 128:
                              half * TPC // 2 + (mt + 1) * 128, :],
                in_=ob,
            )

    # ---------------- emission schedule ----------------
    q_chunk(0)

    # b0/qt0 cache kts while projections stream through `feed`
    pending = (
        [lambda nt=nt: q_chunk(nt) for nt in range(1, 4)]
        + [lambda nt=nt: k_chunk(nt) for nt in range(0, 4)]
        + [lambda mt=mt: v_chunk(mt) for mt in range(0, 16)]
    )
    attn_chunk(0, 0, list(range(0, 16)))
    # force any leftover b0 projections before the new-key half
    while pending:
        feed()
    pending = (
        [lambda nt=nt: q_chunk(nt) for nt in range(4, 8)]
        + [lambda nt=nt: k_chunk(nt) for nt in range(4, 8)]
        + [lambda mt=mt: v_chunk(mt) for mt in range(16, 32)]
    )
    attn_chunk(0, 0, list(range(16, KT_N)))
    for qt in range(1, 4):
        attn_chunk(0, qt, list(range(KT_N)))
    while pending:
        feed()

    a2a_half(0)  # overlaps batch-1 attention

    attn_chunk(1, 0, list(range(KT_N)))
    attn_chunk(1, 1, list(range(KT_N)))
    wo_half(0)  # A2A#1 has completed under the two chunks above
    attn_chunk(1, 2, list(range(KT_N)))
    attn_chunk(1, 3, list(range(KT_N)))

    a2a_half(1)
    wo_half(1)


def build_nc():
    nc = bacc.Bacc(
        "TRN2",
        target_bir_lowering=False,
        debug=False,
        num_devices=NCORES,
    )
    io = {
        "xT": nc.dram_tensor("xT", [D, T], BF16, kind="ExternalInput").ap(),
        "wq": nc.dram_tensor("wq", [D, 128], BF16, kind="ExternalInput").ap(),
        "wk": nc.dram_tensor("wk", [D, 128], BF16, kind="ExternalInput").ap(),
        "wv": nc.dram_tensor("wv", [D, 128], BF16, kind="ExternalInput").ap(),
        "wo": nc.dram_tensor("wo", [D, D], BF16, kind="ExternalInput").ap(),
        "kTc": nc.dram_tensor("kTc", [B, 128, SC], BF16, kind="ExternalInput").ap(),
        "vc": nc.dram_tensor("vc", [B, 2, SC, HD], BF16, kind="ExternalInput").ap(),
        "kT_new": nc.dram_tensor("kT_new", [128, T], BF16, kind="ExternalOutput").ap(),
        "v_new": nc.dram_tensor("v_new", [T, 128], BF16, kind="ExternalOutput").ap(),
        "out_shard": nc.dram_tensor(
            "out_shard", [TPC, D], F32, kind="ExternalOutput"
        ).ap(),
    }
    with tile.TileContext(nc) as tc, ExitStack() as ctx:
        _build_body(ctx, tc, io)
    nc.compile()
    return nc


def make_in_maps(x, kv_k, kv_v, W_Q, W_K, W_V, W_O):
    """Host-side shard/pre-transpose. All fp32 numpy in, bf16 shards out."""
    xT = np.ascontiguousarray(x.reshape(T, D).T).astype(bf)
    wo = W_O.astype(bf)
    in_maps = []
    for i in range(NCORES):
        hsl = slice(2 * i, 2 * i + 2)
        csl = slice(128 * i, 128 * (i + 1))
        # K cache, transposed to [B, 2*64, SC] with heads stacked on axis 1
        kTc = (
            kv_k[:, hsl]                      # [B, 2, SC, HD]
            .transpose(0, 1, 3, 2)            # [B, 2, HD, SC]
            .reshape(B, 128, SC)
            .astype(bf)
        )
        in_maps.append(
            {
                "xT": xT,
                "wq": np.ascontiguousarray(W_Q[:, csl]).astype(bf),
                "wk": np.ascontiguousarray(W_K[:, csl]).astype(bf),
                "wv": np.ascontiguousarray(W_V[:, csl]).astype(bf),
                "wo": wo,
                "kTc": np.ascontiguousarray(kTc),
                "vc": np.ascontiguousarray(kv_v[:, hsl]).astype(bf),
            }
        )
    return in_maps


def assemble(results, kv_k, kv_v):
    """Host-side unshard: rebuild (out, (K, V)) in fp32."""
    out = np.empty((T, D), np.float32)
    for i in range(NCORES):
        sh = results[i]["out_shard"]
        # rows 0:256 = b0 tokens [256i, 256i+256); rows 256:512 = b1 same
        out[i * HTPC:(i + 1) * HTPC] = sh[:HTPC]
        out[S + i * HTPC: S + (i + 1) * HTPC] = sh[HTPC:]
    out = out.reshape(B, S, D)

    k_new = np.empty((B, H, S, HD), np.float32)
    v_new = np.empty((B, H, S, HD), np.float32)
    for i in range(NCORES):
        # kT_new [128, T]: row p = h_local*64 + d, col t = b*S + s
        ktn = np.asarray(results[i]["kT_new"], dtype=np.float32).reshape(
            2, HD, B, S
        )  # [h_local, d, b, s]
        k_new[:, 2 * i:2 * i + 2] = ktn.transpose(2, 0, 3, 1)
        # v_new [T, 128]: row t = b*S + s, col c = h_local*64 + d
        vnn = np.asarray(results[i]["v_new"], dtype=np.float32).reshape(
            B, S, 2, HD
        )
        v_new[:, 2 * i:2 * i + 2] = vnn.transpose(0, 2, 1, 3)

    K = np.concatenate([np.asarray(kv_k, np.float32), k_new], axis=2)
    V = np.concatenate([np.asarray(kv_v, np.float32), v_new], axis=2)
    return out, (K, V)


_NC_CACHE = {}


def get_nc():
    if "nc" not in _NC_CACHE:
        _NC_CACHE["nc"] = build_nc()
    return _NC_CACHE["nc"]


def kernel(x, kv_k, kv_v, W_Q, W_K, W_V, W_O):
    x = np.asarray(x, np.float32)
    kv_k = np.asarray(kv_k, np.float32)
    kv_v = np.asarray(kv_v, np.float32)
    nc = get_nc()
    in_maps = make_in_maps(
        x, kv_k, kv_v,
        np.asarray(W_Q, np.float32), np.asarray(W_K, np.float32),
        np.asarray(W_V, np.float32), np.asarray(W_O, np.float32),
    )
    res = run_bass_kernel_spmd(nc, in_maps, core_ids=list(range(NCORES)))
    return assemble(res.results, kv_k, kv_v)


# revision 16
# speedup vs baseline: 1.1439x; 1.0719x over previous
"""Multi-head attention w/ KV cache, tensor-parallel over 8 TRN2 NeuronCores.

Sharding: heads are split 2-per-core (W_Q/W_K/W_V column shards, KV cache head
shards).  Each core computes Q/K/V projections for its 2 heads, full attention
over the 4096-key axis (2048 cache + 2048 new), then two AllToAlls convert the
head-sharded attention output O^T into a token-sharded full-depth O^T so every
core computes W_O for its own 512-token shard — no all-reduce and no
core-dependent control flow (pure SPMD).

Layout notes (everything transposed so the PE contracts over partitions):
  x^T        [1024, 4096]  d_model on partitions, tokens b-major on free axis
  Q^T/K^T    [128, 4096]   2 heads stacked on partitions (64 rows each)
  scores^T   [keys, q]     K^T stationary, Q^T moving, 2 heads row-packed
  P = exp(s/8) via ScalarE straight out of PSUM, bf16 into SBUF
  O'^T = V_aug^T P^T  with V augmented by a ones column -> row 64 = softmax Z

Scheduling: projection matmul chunks are interleaved into the attention
kt-loop (keeps the PE dense, HAM warm, and hides the projection phase under
ScalarE exp work).  The AllToAll is split in two: token windows are
b-interleaved (window j = b0[256j:256j+256] + b1[256j:256j+256]) so the first
A2A + half the W_O run under batch-1 attention.
"""

import os
import sys

for _p in ("/opt/trn_rl_repo",):
    if os.path.isdir(_p) and _p not in sys.path:
        sys.path.insert(0, _p)

from contextlib import ExitStack

import ml_dtypes
import numpy as np

import concourse.bass as bass
import concourse.tile as tile
from concourse import bacc, mybir
from concourse.bass import ts
from concourse.bass_utils import run_bass_kernel_spmd

BF16 = mybir.dt.bfloat16
F32 = mybir.dt.float32
AF = mybir.ActivationFunctionType

B, S, D = 2, 2048, 1024
H, HD = 16, 64
SC = 2048              # cache length
KT_N = (SC + S) // 128  # 32 key tiles of 128
NCORES = 8
T = B * S              # 4096 tokens, flat b-major
TPC = T // NCORES      # 512 tokens per core
HTPC = TPC // 2        # 256-token half windows
SCALE = 1.0 / np.sqrt(np.float32(HD))

bf = ml_dtypes.bfloat16


def _build_body(ctx: ExitStack, tc, io):
    nc = tc.nc
    xT, wq, wk, wv, wo, kTc, vc = (
        io["xT"], io["wq"], io["wk"], io["wv"], io["wo"], io["kTc"], io["vc"],
    )
    kT_new, v_new, out_shard = io["kT_new"], io["v_new"], io["out_shard"]

    sb = ctx.enter_context(tc.tile_pool(name="sb", bufs=1))
    dramp = ctx.enter_context(tc.tile_pool(name="dramp", bufs=1, space="DRAM"))

    # ---------------- input loads (spread across DMA queues) ----------------
    wq_sb = sb.tile([128, 8, 128], BF16)
    wk_sb = sb.tile([128, 8, 128], BF16)
    wv_sb = sb.tile([128, 8, 128], BF16)
    for kt in range(8):
        nc.gpsimd.dma_start(out=wq_sb[:, kt, :], in_=wq[ts(kt, 128), :])

    kTc_sb = sb.tile([128, B, SC], BF16)
    for b in range(B):
        nc.scalar.dma_start(out=kTc_sb[:, b, :], in_=kTc[b])

    # V augmented with a ones column (col 64 of each 65-wide block).  The
    # cache half arrives host-prepacked (ones included) as one contiguous
    # block per (head, batch); the new half gets ones from the memset and
    # data from the V projection.
    vaug = []  # per local head: [128, B, 32, 65]
    for h in range(2):
        va = sb.tile([128, B, KT_N, 65], BF16, name=f"vaug{h}")
        nc.vector.memset(va, 1.0)
        for b in range(B):
            nc.scalar.dma_start(out=va[:, b, 0:16, :], in_=vc[h, b])
        vaug.append(va)

    xt_sb = sb.tile([128, 8, T], BF16)
    for kt in range(8):
        eng = nc.sync if kt % 2 == 0 else nc.scalar
        eng.dma_start(out=xt_sb[:, kt, :], in_=xT[ts(kt, 128), :])

    for kt in range(8):
        nc.gpsimd.dma_start(out=wk_sb[:, kt, :], in_=wk[ts(kt, 128), :])
        nc.gpsimd.dma_start(out=wv_sb[:, kt, :], in_=wv[ts(kt, 128), :])

    wo_sb = sb.tile([128, 8, D], BF16)
    for kt in range(8):
        nc.gpsimd.dma_start(out=wo_sb[:, kt, :], in_=wo[ts(kt, 128), :])

    # ---------------- PSUM pools (8 banks total, exact fit) ----------------
    # scores: 2 x [128,1024] = 4 banks; proj accumulators: 2 x [128,512] =
    # 2 banks; op0/op1 (per-head O' + Z row, [65,512]) 1 bank each.
    sc_ps = ctx.enter_context(tc.tile_pool(name="sc_ps", bufs=2, space="PSUM"))
    pr_ps = ctx.enter_context(tc.tile_pool(name="pr_ps", bufs=2, space="PSUM"))
    op_ps = ctx.enter_context(tc.tile_pool(name="op_ps", bufs=1, space="PSUM"))
    ptp = ctx.enter_context(tc.tile_pool(name="ptp", bufs=3))
    nrm = ctx.enter_context(tc.tile_pool(name="nrm", bufs=4))
    obp = ctx.enter_context(tc.tile_pool(name="obp", bufs=4))

    qt_sb = sb.tile([128, T], BF16)
    ktn_sb = sb.tile([128, T], BF16)
    ot_sb = sb.tile([128, T], BF16)  # normalized O^T, 2 heads stacked

    # ---------------- projection chunk emitters ----------------
    def q_chunk(nt):
        acc = pr_ps.tile([128, 512], F32, tag="pr", name=f"qacc{nt}")
        for kt in range(8):
            nc.tensor.matmul(
                acc, lhsT=wq_sb[:, kt, :], rhs=xt_sb[:, kt, ts(nt, 512)],
                start=(kt == 0), stop=(kt == 7),
            )
        nc.vector.tensor_copy(qt_sb[:, ts(nt, 512)], acc)

    def k_chunk(nt):
        acc = pr_ps.tile([128, 512], F32, tag="pr", name=f"kacc{nt}")
        for kt in range(8):
            nc.tensor.matmul(
                acc, lhsT=wk_sb[:, kt, :], rhs=xt_sb[:, kt, ts(nt, 512)],
                start=(kt == 0), stop=(kt == 7),
            )
        nc.vector.tensor_copy(ktn_sb[:, ts(nt, 512)], acc)
        if nt == 7:
            nc.sync.dma_start(out=kT_new[:, :], in_=ktn_sb)

    def v_chunk(mt):
        accv = pr_ps.tile([128, 128], F32, tag="pr", name=f"vacc{mt}")
        for kt in range(8):
            nc.tensor.matmul(
                accv, lhsT=xt_sb[:, kt, ts(mt, 128)], rhs=wv_sb[:, kt, :],
                start=(kt == 0), stop=(kt == 7),
            )
        b, j = divmod(mt, 16)
        nc.vector.tensor_copy(vaug[0][:, b, 16 + j, 0:64], accv[:, 0:64])
        nc.vector.tensor_copy(vaug[1][:, b, 16 + j, 0:64], accv[:, 64:128])
        if mt % 16 == 15:
            for h in range(2):
                nc.sync.dma_start(
                    out=v_new[b * S:(b + 1) * S, ts(h, 64)].rearrange(
                        "(j p) d -> p j d", p=128
                    ),
                    in_=vaug[h][:, b, 16:32, 0:64],
                )

    # feeder: proj chunks consumed between attention kt groups
    pending = []

    def feed(n=1):
        for _ in range(n):
            if pending:
                pending.pop(0)()

    cc_in = [dramp.tile([NCORES, 128, HTPC], BF16, name=f"cc_in{half}")
             for half in range(2)]
    cc_out = [dramp.tile([NCORES, 128, HTPC], BF16, name=f"cc_out{half}")
              for half in range(2)]
    otf = [sb.tile([128, NCORES, HTPC], BF16, name=f"otf{half}")
           for half in range(2)]

    # ---------------- attention chunk ----------------
    def attn_chunk(b, qt, kts):
        qlo = b * S + qt * 512
        if kts[0] == 0:
            op = [
                op_ps.tile([65, 512], F32, name=f"op{h}_{b}_{qt}", tag=f"op{h}")
                for h in range(2)
            ]
            attn_chunk.op[(b, qt)] = op
        op = attn_chunk.op[(b, qt)]
        for kt in kts:
            if kt < 16:
                ktile = kTc_sb[:, b, ts(kt, 128)]
            else:
                ktile = ktn_sb[:, b * S + (kt - 16) * 128:
                               b * S + (kt - 15) * 128]
            sc = sc_ps.tile([128, 1024], F32, tag="sc", name=f"sc_{b}_{qt}_{kt}")
            for h in range(2):
                nc.tensor.matmul(
                    sc[:, ts(h, 512)],
                    lhsT=ktile[ts(h, 64), :],
                    rhs=qt_sb[ts(h, 64), qlo:qlo + 512],
                    start=True, stop=True,
                )
            pt = ptp.tile([128, 1024], BF16, tag="pt")
            nc.scalar.activation(pt, sc, AF.Exp, scale=float(SCALE))
            for h in range(2):
                nc.tensor.matmul(
                    attn_chunk.op[(b, qt)][h],
                    lhsT=vaug[h][:, b, kt, :],
                    rhs=pt[:, ts(h, 512)],
                    start=(kt == 0), stop=(kt == KT_N - 1),
                )
            feed()
        if kts[-1] == KT_N - 1:
            # Evacuate O'+Z from PSUM fast (frees the op banks for the next
            # chunk), then normalize from SBUF off the PE critical path.
            for h in range(2):
                oph = attn_chunk.op[(b, qt)][h]
                orw = nrm.tile([65, 512], F32, tag="orw", bufs=3)
                nc.vector.tensor_copy(orw, oph)
                rz = nrm.tile([1, 512], F32, tag="rz")
                nc.vector.reciprocal(rz, orw[64:65, :])
                rzd = dramp.tile([1, 512], F32, tag="rzd", bufs=4)
                nc.sync.dma_start(out=rzd, in_=rz)
                rzb = nrm.tile([64, 512], F32, tag="rzb")
                nc.sync.dma_start(
                    out=rzb,
                    in_=bass.AP(
                        tensor=rzd.tensor,
                        offset=rzd.offset,
                        ap=[[0, 64]] + [list(a) for a in rzd.ap[-1:]],
                    ),
                )
                nc.vector.tensor_mul(
                    ot_sb[ts(h, 64), qlo:qlo + 512], orw[0:64, :], rzb
                )
            for j in (2 * qt, 2 * qt + 1):
                nc.sync.dma_start(
                    out=cc_in[b][j],
                    in_=ot_sb[:, b * S + j * HTPC: b * S + (j + 1) * HTPC],
                )

    attn_chunk.op = {}

    # ---------------- A2A halves + W_O halves ----------------

    def a2a_half(half):
        # cc_in[half] blocks were DMA'd per-chunk inside attn_chunk
        nc.gpsimd.collective_compute(
            "AllToAll",
            mybir.AluOpType.bypass,
            replica_groups=[list(range(NCORES))],
            ins=[cc_in[half].opt()],
            outs=[cc_out[half].opt()],
        )
        for s in range(NCORES):
            # gpsimd queue: naturally ordered after its collective_compute
            nc.gpsimd.dma_start(out=otf[half][:, s, :], in_=cc_out[half][s])

    def wo_half(half):
        # out_shard rows [half*256, half*256+256) = tokens of batch `half`
        for mt in range(2):
            acc = sc_ps.tile([128, 1024], F32, tag="sc", name=f"wacc{half}{mt}")
            for nt in range(2):
                for kt in range(8):
                    nc.tensor.matmul(
                        acc[:, ts(nt, 512)],
                        lhsT=otf[half][:, kt, ts(mt, 128)],
                        rhs=wo_sb[:, kt, ts(nt, 512)],
                        start=(kt == 0), stop=(kt == 7),
                    )
            ob = obp.tile([128, 1024], F32, tag="ob")
            nc.vector.tensor_copy(ob[:, 0:512], acc[:, 0:512])
            nc.scalar.copy(ob[:, 512:1024], acc[:, 512:1024])
            nc.sync.dma_start(
                out=out_shard[half * TPC // 2 + mt * 128:
                              half * TPC // 2 + (mt + 1) * 128, :],
                in_=ob,
            )

    # ---------------- emission schedule ----------------
    q_chunk(0)

    # b0/qt0 cache kts while projections stream through `feed`
    pending = (
        [lambda nt=nt: q_chunk(nt) for nt in range(1, 4)]
        + [lambda nt=nt: k_chunk(nt) for nt in range(0, 4)]
        + [lambda mt=mt: v_chunk(mt) for mt in range(0, 16)]
    )
    attn_chunk(0, 0, list(range(0, 16)))
    # force any leftover b0 projections before the new-key half
    while pending:
        feed()
    pending = (
        [lambda nt=nt: q_chunk(nt) for nt in range(4, 8)]
        + [lambda nt=nt: k_chunk(nt) for nt in range(4, 8)]
        + [lambda mt=mt: v_chunk(mt) for mt in range(16, 32)]
    )
    attn_chunk(0, 0, list(range(16, KT_N)))
    for qt in range(1, 4):
        attn_chunk(0, qt, list(range(KT_N)))
    while pending:
        feed()

    a2a_half(0)  # overlaps batch-1 attention

    attn_chunk(1, 0, list(range(KT_N)))
    attn_chunk(1, 1, list(range(KT_N)))
    wo_half(0)  # A2A#1 has completed under the two chunks above
    attn_chunk(1, 2, list(range(KT_N)))
    attn_chunk(1, 3, list(range(KT_N)))

    a2a_half(1)
    wo_half(1)


def build_nc():
    nc = bacc.Bacc(
        "TRN2",
        target_bir_lowering=False,
        debug=False,
        num_devices=NCORES,
    )
    io = {
        "xT": nc.dram_tensor("xT", [D, T], BF16, kind="ExternalInput").ap(),
        "wq": nc.dram_tensor("wq", [D, 128], BF16, kind="ExternalInput").ap(),
        "wk": nc.dram_tensor("wk", [D, 128], BF16, kind="ExternalInput").ap(),
        "wv": nc.dram_tensor("wv", [D, 128], BF16, kind="ExternalInput").ap(),
        "wo": nc.dram_tensor("wo", [D, D], BF16, kind="ExternalInput").ap(),
        "kTc": nc.dram_tensor("kTc", [B, 128, SC], BF16, kind="ExternalInput").ap(),
        "vc": nc.dram_tensor("vc", [2, B, 128, 16, 65], BF16, kind="ExternalInput").ap(),
        "kT_new": nc.dram_tensor("kT_new", [128, T], BF16, kind="ExternalOutput").ap(),
        "v_new": nc.dram_tensor("v_new", [T, 128], BF16, kind="ExternalOutput").ap(),
        "out_shard": nc.dram_tensor(
            "out_shard", [TPC, D], F32, kind="ExternalOutput"
        ).ap(),
    }
    with tile.TileContext(nc) as tc, ExitStack() as ctx:
        _build_body(ctx, tc, io)
    nc.compile()
    return nc


def make_in_maps(x, kv_k, kv_v, W_Q, W_K, W_V, W_O):
    """Host-side shard/pre-transpose. All fp32 numpy in, bf16 shards out."""
    xT = np.ascontiguousarray(x.reshape(T, D).T).astype(bf)
    wo = W_O.astype(bf)
    in_maps = []
    for i in range(NCORES):
        hsl = slice(2 * i, 2 * i + 2)
        csl = slice(128 * i, 128 * (i + 1))
        # K cache, transposed to [B, 2*64, SC] with heads stacked on axis 1
        kTc = (
            kv_k[:, hsl]                      # [B, 2, SC, HD]
            .transpose(0, 1, 3, 2)            # [B, 2, HD, SC]
            .reshape(B, 128, SC)
            .astype(bf)
        )
        # prepacked V-cache with ones column: [2, B, 128, 16, 65]
        vcp = np.ones((2, B, 128, 16, 65), np.float32)
        vcp[:, :, :, :, 0:64] = (
            kv_v[:, hsl]                       # [B, 2, SC, HD]
            .reshape(B, 2, 16, 128, HD)
            .transpose(1, 0, 3, 2, 4)          # [2, B, 128, 16, HD]
        )
        in_maps.append(
            {
                "xT": xT,
                "wq": np.ascontiguousarray(W_Q[:, csl]).astype(bf),
                "wk": np.ascontiguousarray(W_K[:, csl]).astype(bf),
                "wv": np.ascontiguousarray(W_V[:, csl]).astype(bf),
                "wo": wo,
                "kTc": np.ascontiguousarray(kTc),
                "vc": vcp.astype(bf),
            }
        )
    return in_maps


def assemble(results, kv_k, kv_v):
    """Host-side unshard: rebuild (out, (K, V)) in fp32."""
    out = np.empty((T, D), np.float32)
    for i in range(NCORES):
        sh = results[i]["out_shard"]
        # rows 0:256 = b0 tokens [256i, 256i+256); rows 256:512 = b1 same
        out[i * HTPC:(i + 1) * HTPC] = sh[:HTPC]
        out[S + i * HTPC: S + (i + 1) * HTPC] = sh[HTPC:]
    out = out.reshape(B, S, D)

    k_new = np.empty((B, H, S, HD), np.float32)
    v_new = np.empty((B, H, S, HD), np.float32)
    for i in range(NCORES):
        # kT_new [128, T]: row p = h_local*64 + d, col t = b*S + s
        ktn = np.asarray(results[i]["kT_new"], dtype=np.float32).reshape(
            2, HD, B, S
        )  # [h_local, d, b, s]
        k_new[:, 2 * i:2 * i + 2] = ktn.transpose(2, 0, 3, 1)
        # v_new [T, 128]: row t = b*S + s, col c = h_local*64 + d
        vnn = np.asarray(results[i]["v_new"], dtype=np.float32).reshape(
            B, S, 2, HD
        )
        v_new[:, 2 * i:2 * i + 2] = vnn.transpose(0, 2, 1, 3)

    K = np.concatenate([np.asarray(kv_k, np.float32), k_new], axis=2)
    V = np.concatenate([np.asarray(kv_v, np.float32), v_new], axis=2)
    return out, (K, V)


_NC_CACHE = {}


def get_nc():
    if "nc" not in _NC_CACHE:
        _NC_CACHE["nc"] = build_nc()
    return _NC_CACHE["nc"]


def kernel(x, kv_k, kv_v, W_Q, W_K, W_V, W_O):
    x = np.asarray(x, np.float32)
    kv_k = np.asarray(kv_k, np.float32)
    kv_v = np.asarray(kv_v, np.float32)
    nc = get_nc()
    in_maps = make_in_maps(
        x, kv_k, kv_v,
        np.asarray(W_Q, np.float32), np.asarray(W_K, np.float32),
        np.asarray(W_V, np.float32), np.asarray(W_O, np.float32),
    )
    res = run_bass_kernel_spmd(nc, in_maps, core_ids=list(range(NCORES)))
    return assemble(res.results, kv_k, kv_v)
